# revision 17
# baseline (speedup 1.0000x reference)
"""Multi cross-entropy loss (-mean log p[i, labels[i]]) on 8 Trainium2 cores.

Strategy: data-parallel shard of the N=1048576 rows across 8 cores.  Each
core gathers only its 131072 picked f32 elements from HBM via indirect DMA
(one descriptor per row, ~0.5 MiB moved instead of 128 MiB streamed), takes
log on ScalarE with the fused per-partition accumulator, and reduces to a
single partial sum via a [128,1]x[128,1] matmul.  The host sums the 8
partials and applies -1/N.
"""

import numpy as np

import concourse.bacc as bacc
import concourse.bass as bass
import concourse.mybir as mybir
import concourse.tile as tile
from concourse.bass_utils import run_bass_kernel_spmd

N_CORES = 8
N, C = 1048576, 256
R = N // N_CORES          # rows per core = 131072
P = 128                   # SBUF partitions
F = R // P                # offsets per partition = 1024
NCHUNK = 2                # gather chunks per core
W = F // NCHUNK           # offset columns per chunk = 128

FP32 = mybir.dt.float32
I32 = mybir.dt.int32


def _build_program() -> bass.Bass:
    nc = bacc.Bacc()
    pred = nc.declare_dram_parameter("pred", [R, C], FP32, isOutput=False)
    # idx[:, :F] = labels, idx[:, F:] = row*C base (one DMA so downstream ops
    # depend on a single transfer — walrus limits sync waits per instruction)
    idx = nc.declare_dram_parameter("idx", [P, 2 * F], I32, isOutput=False)
    out = nc.declare_dram_parameter("partial", [1, 1], FP32, isOutput=True)

    with tile.TileContext(nc) as tc:
        with (
            tc.tile_pool(name="main", bufs=1) as mp,
            tc.tile_pool(name="g", bufs=NCHUNK) as gp,
            tc.tile_pool(name="ps", bufs=1, space="PSUM") as pp,
        ):
            idx_t = mp.tile([P, 2 * F], I32)
            ones_t = mp.tile([P, 1], FP32)
            offs_t = mp.tile([P, F], I32)
            acc_t = mp.tile([P, NCHUNK], FP32)

            nc.sync.dma_start(out=idx_t[:], in_=idx[:])
            nc.vector.memset(ones_t[:], 1.0)

            # offs[p, j] = (p*F + j)*C + label[p*F + j].  C=256 and label<256,
            # so this is a bitwise OR of disjoint bit ranges — exact on the
            # DVE int path (a wide int add would round through fp32).
            nc.vector.tensor_tensor(
                out=offs_t[:],
                in0=idx_t[:, :F],
                in1=idx_t[:, F:],
                op=mybir.AluOpType.bitwise_or,
            )

            for k in range(NCHUNK):
                g_t = gp.tile([P, W], FP32, tag="g")
                gl_t = gp.tile([P, W], FP32, tag="gl")
                nc.gpsimd.indirect_dma_start(
                    out=g_t[:],
                    out_offset=None,
                    in_=pred[:],
                    in_offset=bass.IndirectOffsetOnAxis(
                        ap=offs_t[:, k * W : (k + 1) * W], axis=1
                    ),
                )
                # gl = ln(g); acc[:, k] = sum_j gl[:, j]
                nc.scalar.activation(
                    out=gl_t[:],
                    in_=g_t[:],
                    func=mybir.ActivationFunctionType.Ln,
                    accum_out=acc_t[:, k : k + 1],
                )

            rowsum_t = mp.tile([P, 1], FP32)
            nc.vector.tensor_reduce(
                out=rowsum_t[:],
                in_=acc_t[:],
                axis=mybir.AxisListType.X,
                op=mybir.AluOpType.add,
            )
            ps_t = pp.tile([1, 1], FP32)
            nc.tensor.matmul(
                out=ps_t[:], lhsT=rowsum_t[:], rhs=ones_t[:], start=True, stop=True
            )
            res_t = mp.tile([1, 1], FP32)
            nc.vector.tensor_copy(out=res_t[:], in_=ps_t[:])
            nc.sync.dma_start(out=out[:], in_=res_t[:])
    nc.compile()  # bacc passes: split multi-waits into event sems, etc.
    return nc


def _in_maps(predicts: np.ndarray, labels: np.ndarray) -> list[dict[str, np.ndarray]]:
    base = (np.arange(R, dtype=np.int32) * C).reshape(P, F)
    labels_i32 = np.ascontiguousarray(labels.reshape(N).astype(np.int32))
    maps = []
    for s in range(N_CORES):
        lab_s = labels_i32[s * R : (s + 1) * R].reshape(P, F)
        maps.append(
            {
                "pred": predicts[s * R : (s + 1) * R],
                "idx": np.concatenate([lab_s, base], axis=1),
            }
        )
    return maps


def _run(predicts: np.ndarray, labels: np.ndarray, **run_kwargs):
    predicts = np.asarray(predicts)
    labels = np.asarray(labels)
    assert predicts.shape == (N, C) and predicts.dtype == np.float32
    nc = _build_program()
    res = run_bass_kernel_spmd(
        nc, _in_maps(predicts, labels), list(range(N_CORES)), **run_kwargs
    )
    total = float(sum(float(r["partial"][0, 0]) for r in res.results))
    loss = np.float32(-total / N)
    return loss, res


def kernel(predicts: np.ndarray, labels: np.ndarray) -> np.ndarray:
    loss, _ = _run(predicts, labels)
    return np.asarray(loss, dtype=np.float32)
